# revision 7
# baseline (speedup 1.0000x reference)
"""Trainium2 Bass kernel for nn_ClusterNet (3-layer linear GraphSAGE + max-pool + log_softmax).

Strategy
--------
The network is linear up to the final log_softmax:
    h3 = sum_{k=0..3} (M^k xt) Ct_k,   xt = [x | 1]  (bias column rides along),
where M = D^-1 A is the mean-aggregation operator and Ct_k are host-folded
products of the small weight matrices (weights are replicated; folding them is
constant preprocessing).

Device work (8 NeuronCores, graph-sharded by batch-id ranges so aggregation
output rows and pooling are core-local):
  - 3 rounds of "apply M": gather neighbor rows via nc.gpsimd.dma_gather from
    f32 tables (int16 indices -> 4 table windows). Nodes are sorted by
    in-degree (desc) and packed into 128-row groups; each group's slot count
    K is the max degree in the group, so padding is ~4% (vs 21% for
    power-of-two degree buckets). A strided tree-add reduces each node's K
    slots, then a recip scale. Gather calls alternate between 2 SWDGE queues.
  - window partials are recombined by a tiny add/scale launch; the host only
    PERMUTES rows between launches (no arithmetic on tensor data).
  - tail launch: h3 = sum_k y_k @ Ct_k via PE (transpose + accumulating
    matmuls). Local rows are laid out so each graph starts at a 128-aligned
    offset (gmax128 stride): the block->graph map is static and shared by all
    cores; a per-core additive mask (-1e30 on pad rows) makes the per-block
    partition-wise running max correct. One PE transpose + free-dim tree-max
    per graph, then logits and log_softmax on-device. Output [8 graphs, 8]
    per core, assembled on host.
"""
import os
import sys

sys.path.insert(0, '/opt/trn_rl_repo')

import numpy as np

import concourse.bass as bass
import concourse.bacc as bacc
import concourse.tile as tile
import concourse.mybir as mybir
from concourse import bass_utils
from concourse.masks import make_identity

NCORES = 8
F = 64            # table row width (f32, 256B rows; cols 0..32 used)
FIN = 33          # x(32) + ones column
NW = 4            # table windows (int16 index limit)
CALL_MAX = 8192   # max indices per dma_gather call
TRACE = bool(os.environ.get("KERNEL_TRACE"))

LAST_EXEC_NS = []   # per-launch exec_time_ns when KERNEL_TRACE=1

if TRACE:
    # NTFF profiling shim: the image's antenv lacks axon_hooks, so register it
    # ourselves (dev-only; graders run with KERNEL_TRACE unset).
    import types

    if "antenv.axon_hooks" not in sys.modules:
        _m = types.ModuleType("antenv.axon_hooks")
        _m._hook = None
        _m.set_axon_ntff_profile_hook = lambda h: setattr(_m, "_hook", h)
        _m.get_axon_ntff_profile_hook = lambda: _m._hook
        sys.modules["antenv.axon_hooks"] = _m
        try:
            from trn_agent_boot.trn_boot import _ntff_profile_via_ctypes
            _m._hook = _ntff_profile_via_ctypes("/opt/axon/libaxon_pjrt.so")
        except Exception:
            _m._hook = None
    bass_utils.upload_artifacts = lambda tmpdir: f"local:{tmpdir}"

_prog_cache = {}


def _roundup(a, b):
    return (a + b - 1) // b * b


# ----------------------------------------------------------------- host plan
class Plan:
    pass


def build_plan(edge_index, batch, N, G):
    pl = Plan()
    src = np.asarray(edge_index[0], np.int64)
    dst = np.asarray(edge_index[1], np.int64)
    batch = np.asarray(batch, np.int64)
    gpc = G // NCORES  # graphs per core

    core_of_graph = np.arange(G) // gpc
    node_core = core_of_graph[batch]                      # [N]
    n0 = np.searchsorted(batch, np.arange(0, G, gpc))     # core node range start
    n1 = np.append(n0[1:], N)
    pl.n0, pl.n1, pl.gpc = n0, n1, gpc

    # graph-aligned local row layout: graph gi of a core starts at local row
    # (gi % gpc) * gmax128
    gstart = np.searchsorted(batch, np.arange(G))
    gsizes = np.bincount(batch, minlength=G)
    gmax128 = int(_roundup(max(int(gsizes.max()), 1), 128))
    NLpad = gpc * gmax128
    pl.gmax128, pl.NLpad = gmax128, NLpad

    lrow = (batch % gpc) * gmax128 + (np.arange(N) - gstart[batch])  # [N]
    pl.lrow = lrow
    pl.lrow2node = np.full((NCORES, NLpad), -1, np.int64)
    for c in range(NCORES):
        nn = np.arange(n0[c], n1[c])
        pl.lrow2node[c, lrow[nn]] = nn

    deg = np.bincount(dst, minlength=N)
    recip = np.where(deg > 0, 1.0 / np.maximum(deg, 1), 0.0).astype(np.float32)

    TL = NCORES * NLpad
    WROW = _roundup((TL + NW - 1) // NW, 128)
    assert WROW + 1 <= 32767, WROW
    pl.WROW = WROW

    # logical table row of node n
    tl = node_core[np.arange(N)] * NLpad + lrow
    pl.tl = tl

    src_w = tl[src] // WROW          # window of each edge's source
    src_li = tl[src] - src_w * WROW  # local row within window

    # per-core local edges
    dst_core = node_core[dst]
    ecore = [np.nonzero(dst_core == c)[0] for c in range(NCORES)]

    # per core, per window: CSR of edges grouped by local dst row, and the
    # node order sorted by in-window degree (desc). Groups of 128 nodes in
    # that order share one slot count K = max degree in the group (over cores).
    pl.win_nodes = []   # [c][w] -> local dst rows with deg_w>0, degree-sorted
    pl.win_adj = []     # [c][w] -> (uniq, starts, liw) CSR arrays
    profs = [[None] * NW for _ in range(NCORES)]
    for c in range(NCORES):
        e = ecore[c]
        ld = lrow[dst[e]]
        w = src_w[e]
        li = src_li[e]
        pl.win_nodes.append([])
        pl.win_adj.append([])
        for wi in range(NW):
            m = w == wi
            ldw, liw = ld[m], li[m]
            # sort by (dst row, src row): src-sorted slot runs improve DRAM
            # locality of the gather reads
            order = np.lexsort((liw, ldw))
            ldw, liw = ldw[order], liw[order]
            uniq, counts = np.unique(ldw, return_counts=True)
            # sort nodes by degree desc (stable in row id)
            no = np.argsort(-counts, kind='stable')
            pl.win_nodes[c].append(uniq[no])
            profs[c][wi] = counts[no]
            starts = np.concatenate([[0], np.cumsum(counts)])
            pl.win_adj[c].append((uniq, starts, liw))

    # per window: group count and per-group K (max over cores)
    pl.Ks = []          # [w] -> list of K per group
    for wi in range(NW):
        mx = max(len(profs[c][wi]) for c in range(NCORES))
        ng = (mx + 127) // 128
        P = np.zeros((NCORES, ng * 128), np.int64)
        for c in range(NCORES):
            P[c, :len(profs[c][wi])] = profs[c][wi]
        Ks = [int(P[:, g * 128:(g + 1) * 128].max()) for g in range(ng)]
        pl.Ks.append(Ks)

    # static call list: runs of equal-K groups, capped at CALL_MAX idxs
    calls = []
    sbase = 0
    icol = 0
    for wi in range(NW):
        Ks = pl.Ks[wi]
        g = 0
        while g < len(Ks):
            K = Ks[g]
            assert 1 <= K and 128 * K <= CALL_MAX, K
            cap = CALL_MAX // (128 * K)
            run = 1
            while run < cap and g + run < len(Ks) and Ks[g + run] == K:
                run += 1
            ni = run * K * 128
            calls.append(dict(w=wi, K=K, G=run, g0=g, sbase=sbase,
                              icol=icol, ni=ni))
            sbase += run * 128
            icol += ni // 16
            g += run
    pl.calls = calls
    pl.STOT = sbase
    pl.IDXC = icol

    # per-core idx buffer + recip buffer + S-row -> local-row map
    pl.idxbuf = np.full((NCORES, 128, pl.IDXC), WROW, np.int16)
    pl.recbuf = np.zeros((NCORES, 128, pl.STOT // 128), np.float32)
    pl.srow_node = np.full((NCORES, pl.STOT), -1, np.int64)  # local row or -1
    for c in range(NCORES):
        for call in calls:
            wi, K, Gc, g0 = call['w'], call['K'], call['G'], call['g0']
            nodes = pl.win_nodes[c][wi]
            uniq, starts, liw = pl.win_adj[c][wi]
            rank0 = g0 * 128
            r = max(0, min(len(nodes) - rank0, Gc * 128))
            L = np.full((Gc * K * 128,), pl.WROW, np.int16)  # default zero row
            if r > 0:
                nb = nodes[rank0:rank0 + r]
                t = np.arange(r)
                pl.srow_node[c, call['sbase'] + t] = nb
                ui = np.searchsorted(uniq, nb)
                d = (starts[ui + 1] - starts[ui]).astype(np.int64)
                assert d.max(initial=0) <= K
                tot = int(d.sum())
                tt = np.repeat(t, d)
                off = np.concatenate([[0], np.cumsum(d)[:-1]])
                jj = np.arange(tot) - np.repeat(off, d)
                L[((tt // 128) * K + jj) * 128 + (tt % 128)] = \
                    liw[np.repeat(starts[ui], d) + jj]
            ni = call['ni']
            Lw = L.reshape(ni // 16, 16).T  # [16, ni/16]
            pl.idxbuf[c, :, call['icol']:call['icol'] + ni // 16] = \
                np.tile(Lw, (8, 1))
    # real recip values
    for c in range(NCORES):
        rows = np.nonzero(pl.srow_node[c] >= 0)[0]
        gnodes = pl.lrow2node[c, pl.srow_node[c, rows]]
        assert (gnodes >= 0).all()
        pl.recbuf[c, rows % 128, rows // 128] = recip[gnodes]

    # per-core pooling mask: 0 on real rows, -1e30 on pad rows
    NB = NLpad // 128
    pl.poolmask = np.full((NCORES, 128, NB), -1e30, np.float32)
    for c in range(NCORES):
        rr = np.nonzero(pl.lrow2node[c] >= 0)[0]
        pl.poolmask[c, rr % 128, rr // 128] = 0.0
    return pl


def plan_from_inputs(edge_index, batch):
    return build_plan(edge_index, batch, batch.shape[0], int(batch.max()) + 1)


# ----------------------------------------------------------- device programs
def prog_agg(pl):
    key = ('agg', pl.STOT, pl.IDXC, len(pl.calls), pl.WROW)
    if key in _prog_cache:
        return _prog_cache[key]
    nc = bacc.Bacc("TRN2", target_bir_lowering=False, debug=False,
                   num_devices=NCORES, num_swdge_queues=4)
    tabs = [nc.dram_tensor(f"tab{w}", (pl.WROW + 1, F), mybir.dt.float32,
                           kind="ExternalInput").ap() for w in range(NW)]
    idx = nc.dram_tensor("idx", (128, pl.IDXC), mybir.dt.int16,
                         kind="ExternalInput").ap()
    rec = nc.dram_tensor("rec", (128, pl.STOT // 128), mybir.dt.float32,
                         kind="ExternalInput").ap()
    S = nc.dram_tensor("S", (pl.STOT, F), mybir.dt.float32,
                       kind="ExternalOutput").ap()

    with tile.TileContext(nc) as tc:
        with tc.tile_pool(name="io", bufs=1) as iop, \
             tc.tile_pool(name="g", bufs=6) as gp, \
             tc.tile_pool(name="st", bufs=4) as stp:
            idx_t = iop.tile([128, pl.IDXC], mybir.dt.int16)
            # chunked load so the first gathers start as soon as their index
            # columns have landed (Tile range-tracks the subtile writes)
            NCH = 8
            cw = _roundup((pl.IDXC + NCH - 1) // NCH, 16)
            for h in range(NCH):
                lo = h * cw
                hi = min(pl.IDXC, lo + cw)
                if hi > lo:
                    nc.sync.dma_start(out=idx_t[:, lo:hi], in_=idx[:, lo:hi])
            rec_t = iop.tile([128, pl.STOT // 128], mybir.dt.float32)
            nc.sync.dma_start(out=rec_t[:], in_=rec[:, :])
            for ci, call in enumerate(pl.calls):
                wi, K, Gc, ni = call['w'], call['K'], call['G'], call['ni']
                icol, sbase = call['icol'], call['sbase']
                t = gp.tile([128, Gc * K * F], mybir.dt.float32, tag="g")
                nc.gpsimd.dma_gather(
                    out_ap=t[:].rearrange("p (b f) -> p b f", f=F),
                    in_ap=tabs[wi][:],
                    idxs_ap=idx_t[:, icol:icol + ni // 16],
                    num_idxs=ni, num_idxs_reg=ni, elem_size=F,
                    single_packet=False, queue_num=ci % 4)
                tv = t[:].rearrange("p (g k f) -> p g k f", g=Gc, k=K)
                kk = K
                while kk > 1:
                    h = kk // 2
                    nc.vector.tensor_add(
                        out=tv[:, :, :h, :],
                        in0=tv[:, :, :h, :],
                        in1=tv[:, :, h:2 * h, :])
                    if kk % 2 == 1:
                        nc.vector.tensor_add(
                            out=tv[:, :, 0, :],
                            in0=tv[:, :, 0, :],
                            in1=tv[:, :, kk - 1, :])
                    kk = h
                stg = stp.tile([128, Gc * F], mybir.dt.float32, tag="st")
                rbc = rec_t[:, sbase // 128:sbase // 128 + Gc]
                nc.vector.tensor_mul(
                    out=stg[:].rearrange("p (g f) -> p g f", f=F),
                    in0=tv[:, :, 0, :],
                    in1=rbc.unsqueeze(2).broadcast_to([128, Gc, F]))
                nc.sync.dma_start(
                    out=S[sbase:sbase + Gc * 128, :].rearrange(
                        "(g p) f -> p g f", p=128),
                    in_=stg[:].rearrange("p (g f) -> p g f", f=F))
    nc.compile()
    _prog_cache[key] = nc
    return nc


def prog_comb(pl):
    key = ('comb', pl.NLpad)
    if key in _prog_cache:
        return _prog_cache[key]
    nc = bacc.Bacc("TRN2", target_bir_lowering=False, debug=False,
                   num_devices=NCORES)
    NB = pl.NLpad // 128
    Ss = [nc.dram_tensor(f"S{w}", (pl.NLpad, F), mybir.dt.float32,
                         kind="ExternalInput").ap() for w in range(NW)]
    y = nc.dram_tensor("y", (pl.NLpad, F), mybir.dt.float32,
                       kind="ExternalOutput").ap()
    with tile.TileContext(nc) as tc:
        with tc.tile_pool(name="p", bufs=1) as pp:
            ts = []
            for w in range(NW):
                t = pp.tile([128, NB * F], mybir.dt.float32, tag=f"s{w}")
                nc.sync.dma_start(
                    out=t[:].rearrange("p (b f) -> p b f", f=F),
                    in_=Ss[w][:, :].rearrange("(b p) f -> p b f", p=128))
                ts.append(t)
            nc.vector.tensor_add(out=ts[0][:], in0=ts[0][:], in1=ts[1][:])
            nc.vector.tensor_add(out=ts[2][:], in0=ts[2][:], in1=ts[3][:])
            nc.vector.tensor_add(out=ts[0][:], in0=ts[0][:], in1=ts[2][:])
            nc.sync.dma_start(
                out=y[:, :].rearrange("(b p) f -> p b f", p=128),
                in_=ts[0][:].rearrange("p (b f) -> p b f", f=F))
    nc.compile()
    _prog_cache[key] = nc
    return nc


def prog_tail(pl):
    key = ('tail', pl.NLpad, pl.gmax128, pl.gpc)
    if key in _prog_cache:
        return _prog_cache[key]
    nc = bacc.Bacc("TRN2", target_bir_lowering=False, debug=False,
                   num_devices=NCORES)
    NB = pl.NLpad // 128
    BPG = pl.gmax128 // 128  # blocks per graph
    GPC = pl.gpc
    Ss = [nc.dram_tensor(f"S{w}", (pl.NLpad, F), mybir.dt.float32,
                         kind="ExternalInput").ap() for w in range(NW)]
    ys = [nc.dram_tensor(f"y{k}", (pl.NLpad, F), mybir.dt.float32,
                         kind="ExternalInput").ap() for k in range(3)]
    C = nc.dram_tensor("C", (4 * F, F), mybir.dt.float32,
                       kind="ExternalInput").ap()
    Wo = nc.dram_tensor("Wo", (F, 8), mybir.dt.float32,
                        kind="ExternalInput").ap()
    bo = nc.dram_tensor("bo", (8, 8), mybir.dt.float32,
                        kind="ExternalInput").ap()
    msk = nc.dram_tensor("msk", (128, NB), mybir.dt.float32,
                         kind="ExternalInput").ap()
    out = nc.dram_tensor("out", (8, 8), mybir.dt.float32,
                         kind="ExternalOutput").ap()

    with tile.TileContext(nc) as tc:
        with tc.tile_pool(name="big", bufs=1) as bigp, \
             tc.tile_pool(name="wk", bufs=3) as wk, \
             tc.tile_pool(name="ps", bufs=2, space="PSUM") as ps:
            Ct = bigp.tile([128, 2 * F], mybir.dt.float32)  # [[C0;C1],[C2;C3]]
            nc.sync.dma_start(
                out=Ct[:].rearrange("p (k f) -> p k f", f=F),
                in_=C[:, :].rearrange("(k p) f -> p k f", p=128))
            ident = bigp.tile([128, 128], mybir.dt.float32)
            make_identity(nc, ident[:])
            Wot = bigp.tile([F, 8], mybir.dt.float32)
            nc.sync.dma_start(out=Wot[:], in_=Wo[:, :])
            bot = bigp.tile([8, 8], mybir.dt.float32)
            nc.sync.dma_start(out=bot[:], in_=bo[:, :])
            mskt = bigp.tile([128, NB], mybir.dt.float32)
            nc.sync.dma_start(out=mskt[:], in_=msk[:, :])
            acc = bigp.tile([128, GPC * F], mybir.dt.float32)
            nc.vector.memset(acc[:], -1e30)

            # h3 blocks: pack [y0|y1] and [y2|y3] pairs so one [128,128]
            # transpose + one matmul with stacked C rows handles two terms.
            CB = 28
            for c0 in range(0, NB, CB):
                cb = min(CB, NB - c0)
                rows = slice(c0 * 128, (c0 + cb) * 128)
                p01 = wk.tile([128, CB * 128], mybir.dt.float32, tag="p01")
                p23 = wk.tile([128, CB * 128], mybir.dt.float32, tag="p23")
                pv01 = p01[:].rearrange("p (b t) -> p b t", t=128)
                pv23 = p23[:].rearrange("p (b t) -> p b t", t=128)
                nc.sync.dma_start(
                    out=pv01[:, :cb, 0:F],
                    in_=ys[0][rows, :].rearrange("(b p) f -> p b f", p=128))
                nc.sync.dma_start(
                    out=pv01[:, :cb, F:128],
                    in_=ys[1][rows, :].rearrange("(b p) f -> p b f", p=128))
                nc.sync.dma_start(
                    out=pv23[:, :cb, 0:F],
                    in_=ys[2][rows, :].rearrange("(b p) f -> p b f", p=128))
                nc.sync.dma_start(
                    out=pv23[:, :cb, F:128],
                    in_=Ss[0][rows, :].rearrange("(b p) f -> p b f", p=128))
                for w in range(1, NW):
                    t = wk.tile([128, CB * F], mybir.dt.float32, tag="sw")
                    nc.sync.dma_start(
                        out=t[:, :cb * F].rearrange("p (b f) -> p b f", f=F),
                        in_=Ss[w][rows, :].rearrange("(b p) f -> p b f", p=128))
                    nc.vector.tensor_add(
                        out=pv23[:, :cb, F:128], in0=pv23[:, :cb, F:128],
                        in1=t[:, :cb * F].rearrange("p (b f) -> p b f", f=F))
                for blk in range(cb):
                    hp = ps.tile([128, F], mybir.dt.float32, space="PSUM",
                                 tag="hp")
                    for half, pv in ((0, pv01), (1, pv23)):
                        tp = ps.tile([128, 128], mybir.dt.float32,
                                     space="PSUM", tag="tp")
                        nc.tensor.transpose(out=tp[:], in_=pv[:, blk, :],
                                            identity=ident[:])
                        ykT = wk.tile([128, 128], mybir.dt.float32, tag="ykT")
                        nc.scalar.copy(out=ykT[:], in_=tp[:])
                        nc.tensor.matmul(
                            out=hp[:], lhsT=ykT[:],
                            rhs=Ct[:, half * F:(half + 1) * F],
                            start=(half == 0), stop=(half == 1))
                    b = c0 + blk
                    h3b = wk.tile([128, F], mybir.dt.float32, tag="h3b")
                    # mask pad rows to -1e30 while copying out of PSUM
                    nc.vector.tensor_scalar(
                        out=h3b[:], in0=hp[:], scalar1=mskt[:, b:b + 1],
                        scalar2=None, op0=mybir.AluOpType.add)
                    g = b // BPG
                    nc.vector.tensor_tensor(
                        out=acc[:, g * F:(g + 1) * F],
                        in0=acc[:, g * F:(g + 1) * F],
                        in1=h3b[:], op=mybir.AluOpType.max)

            # per graph: transpose [128,F] -> [F,128], tree-max over free dim
            pooledT = wk.tile([F, 8], mybir.dt.float32, tag="pt")
            for g in range(GPC):
                tp2 = ps.tile([F, 128], mybir.dt.float32, space="PSUM",
                              tag="tp")
                nc.tensor.transpose(out=tp2[:], in_=acc[:, g * F:(g + 1) * F],
                                    identity=ident[:])
                pc = wk.tile([F, 128], mybir.dt.float32, tag="pc")
                nc.scalar.copy(out=pc[:], in_=tp2[:])
                cc = 128
                while cc > 1:
                    h = cc // 2
                    nc.vector.tensor_tensor(
                        out=pc[:, :h], in0=pc[:, :h],
                        in1=pc[:, h:2 * h], op=mybir.AluOpType.max)
                    cc = h
                nc.vector.tensor_copy(out=pooledT[:, g:g + 1], in_=pc[:, :1])
            # logits = pooled @ Wo + bo
            lg = ps.tile([8, 8], mybir.dt.float32, space="PSUM", tag="lg")
            nc.tensor.matmul(out=lg[:], lhsT=pooledT[:], rhs=Wot[:],
                             start=True, stop=True)
            lgs = wk.tile([8, 8], mybir.dt.float32, tag="lgs")
            nc.vector.tensor_add(out=lgs[:], in0=lg[:], in1=bot[:])
            # log_softmax along free dim
            mx = wk.tile([8, 1], mybir.dt.float32, tag="mx")
            nc.vector.tensor_reduce(out=mx[:], in_=lgs[:],
                                    axis=mybir.AxisListType.X,
                                    op=mybir.AluOpType.max)
            nc.vector.tensor_scalar(out=lgs[:], in0=lgs[:], scalar1=mx[:, :1],
                                    scalar2=None,
                                    op0=mybir.AluOpType.subtract)
            ex = wk.tile([8, 8], mybir.dt.float32, tag="ex")
            nc.scalar.activation(out=ex[:], in_=lgs[:],
                                 func=mybir.ActivationFunctionType.Exp)
            sm = wk.tile([8, 1], mybir.dt.float32, tag="sm")
            nc.vector.tensor_reduce(out=sm[:], in_=ex[:],
                                    axis=mybir.AxisListType.X,
                                    op=mybir.AluOpType.add)
            lns = wk.tile([8, 1], mybir.dt.float32, tag="lns")
            nc.scalar.activation(out=lns[:], in_=sm[:],
                                 func=mybir.ActivationFunctionType.Ln)
            nc.vector.tensor_scalar(out=lgs[:], in0=lgs[:], scalar1=lns[:, :1],
                                    scalar2=None,
                                    op0=mybir.AluOpType.subtract)
            nc.sync.dma_start(out=out[:, :], in_=lgs[:])
    nc.compile()
    _prog_cache[key] = nc
    return nc


# ----------------------------------------------------------------- execution
def _run(nc, in_maps):
    res = bass_utils.run_bass_kernel_spmd(nc, in_maps,
                                          core_ids=list(range(NCORES)),
                                          trace=TRACE)
    if TRACE:
        LAST_EXEC_NS.append(res.exec_time_ns)
    return res.results


def _tables_from_y(pl, ylocal):
    """ylocal: [NCORES, NLpad, F] -> 4 window tables [WROW+1, F] (shared
    logical row space; per-core identical)."""
    TLrows = NCORES * pl.NLpad
    flat = np.zeros((NW * (pl.WROW + 1), F), np.float32)
    full = ylocal.reshape(TLrows, F)
    for w in range(NW):
        lo = w * pl.WROW
        hi = min(lo + pl.WROW, TLrows)
        flat[w * (pl.WROW + 1):w * (pl.WROW + 1) + (hi - lo)] = full[lo:hi]
    return [flat[w * (pl.WROW + 1):(w + 1) * (pl.WROW + 1)] for w in range(NW)]


def _align_partials(pl, Sout):
    """Sout: [NCORES, STOT, F] window-ordered partial sums -> aligned
    [NW, NCORES, NLpad, F] (host permutation only)."""
    out = np.zeros((NW, NCORES, pl.NLpad, F), np.float32)
    # S rows -> (window, local row) via call list
    for call in pl.calls:
        wi = call['w']
        rows = np.arange(call['sbase'], call['sbase'] + call['G'] * 128)
        for c in range(NCORES):
            nodes = pl.srow_node[c, rows]
            m = nodes >= 0
            out[wi, c, nodes[m]] = Sout[c, rows[m]]
    return out


def kernel(**inputs):
    x = np.asarray(inputs['x'], np.float32)
    edge_index = np.asarray(inputs['edge_index'])
    batch = np.asarray(inputs['batch'])
    N = x.shape[0]
    G = int(batch.max()) + 1
    pl = build_plan(edge_index, batch, N, G)

    # folded coefficient matrices (weights only)
    Wl = [np.asarray(inputs[f'Wl{i}'], np.float64) for i in range(3)]
    Wr = [np.asarray(inputs[f'Wr{i}'], np.float64) for i in range(3)]
    bl = [np.asarray(inputs[f'bl{i}'], np.float64) for i in range(3)]
    C0 = Wr[0] @ Wr[1] @ Wr[2]
    C1 = Wr[0] @ Wr[1] @ Wl[2] + Wr[0] @ Wl[1] @ Wr[2] + Wl[0] @ Wr[1] @ Wr[2]
    C2 = Wr[0] @ Wl[1] @ Wl[2] + Wl[0] @ Wr[1] @ Wl[2] + Wl[0] @ Wl[1] @ Wr[2]
    C3 = Wl[0] @ Wl[1] @ Wl[2]
    d0 = bl[0] @ Wr[1] @ Wr[2] + bl[1] @ Wr[2] + bl[2]
    d1 = bl[0] @ (Wr[1] @ Wl[2] + Wl[1] @ Wr[2]) + bl[1] @ Wl[2]
    d2 = bl[0] @ Wl[1] @ Wl[2]
    d3 = np.zeros(64)
    Cs = []
    for Cm, dv in [(C0, d0), (C1, d1), (C2, d2), (C3, d3)]:
        Cp = np.zeros((F, F), np.float32)
        Cp[:32] = Cm
        Cp[32] = dv
        Cs.append(Cp)
    Cstack = np.concatenate(Cs, axis=0)  # [4*64, 64]

    # y0 local: [NCORES, NLpad, 64], cols 0..31 = x, col 32 = 1 (real rows)
    y0 = np.zeros((NCORES, pl.NLpad, F), np.float32)
    for c in range(NCORES):
        rr = np.nonzero(pl.lrow2node[c] >= 0)[0]
        y0[c, rr, :32] = x[pl.lrow2node[c, rr]]
        y0[c, rr, 32] = 1.0

    nc_agg = prog_agg(pl)
    nc_comb = prog_comb(pl)

    ys = [y0]
    ycur = y0
    for _ in range(3):
        tabs = _tables_from_y(pl, ycur)
        in_maps = []
        for c in range(NCORES):
            m = {f"tab{w}": tabs[w] for w in range(NW)}
            m["idx"] = pl.idxbuf[c]
            m["rec"] = pl.recbuf[c]
            in_maps.append(m)
        res = _run(nc_agg, in_maps)
        Sout = np.stack([res[c]["S"] for c in range(NCORES)])
        parts = _align_partials(pl, Sout)
        if len(ys) < 3:
            in_maps = [{f"S{w}": parts[w, c] for w in range(NW)}
                       for c in range(NCORES)]
            res = _run(nc_comb, in_maps)
            ycur = np.stack([res[c]["y"] for c in range(NCORES)])
            ys.append(ycur)
        else:
            last_parts = parts
            break

    nc_tail = prog_tail(pl)
    bo = np.asarray(inputs['b_out'], np.float32)[None, :].repeat(8, axis=0)
    Wo = np.zeros((F, 8), np.float32)
    Wo[:] = np.asarray(inputs['W_out'], np.float32)
    in_maps = []
    for c in range(NCORES):
        m = {f"S{w}": last_parts[w, c] for w in range(NW)}
        for k in range(3):
            m[f"y{k}"] = ys[k][c]
        m["C"] = Cstack
        m["Wo"] = Wo
        m["bo"] = bo
        m["msk"] = pl.poolmask[c]
        in_maps.append(m)
    res = _run(nc_tail, in_maps)

    gpc = pl.gpc
    out = np.zeros((G, 8), np.float32)
    for c in range(NCORES):
        out[c * gpc:(c + 1) * gpc] = res[c]["out"]
    return out


# revision 9
# speedup vs baseline: 1.0350x; 1.0350x over previous
"""Trainium2 Bass kernel for nn_ClusterNet (3-layer linear GraphSAGE + max-pool + log_softmax).

Strategy
--------
The network is linear up to the final log_softmax:
    h3 = sum_{k=0..3} (M^k xt) Ct_k,   xt = [x | 1]  (bias column rides along),
where M = D^-1 A is the mean-aggregation operator and Ct_k are host-folded
products of the small weight matrices (weights are replicated; folding them is
constant preprocessing).

Device work (8 NeuronCores, graph-sharded by batch-id ranges so aggregation
output rows and pooling are core-local):
  - 3 rounds of "apply M": gather neighbor rows via nc.gpsimd.dma_gather from
    f32 tables (int16 indices -> 4 table windows). Nodes are sorted by
    in-degree (desc) and packed into 128-row groups; each group's slot count
    K is the max degree in the group, so padding is ~4% (vs 21% for
    power-of-two degree buckets). A strided tree-add reduces each node's K
    slots, then a recip scale. Gather calls rotate across all 4 SWDGE queues
    so Q7 descriptor generation and the SDMA transfers pipeline (~3.2 ns/idx
    vs ~9.3 single-queue).
  - window partials are recombined by a tiny add/scale launch; the host only
    PERMUTES rows between launches (no arithmetic on tensor data).
  - tail launch: h3 = sum_k y_k @ Ct_k via PE (transpose + accumulating
    matmuls). Local rows are laid out so each graph starts at a 128-aligned
    offset (gmax128 stride): the block->graph map is static and shared by all
    cores; a per-core additive mask (-1e30 on pad rows) makes the per-block
    partition-wise running max correct. One PE transpose + free-dim tree-max
    per graph, then logits and log_softmax on-device. Output [8 graphs, 8]
    per core, assembled on host.
"""
import os
import sys

sys.path.insert(0, '/opt/trn_rl_repo')

import numpy as np

import concourse.bass as bass
import concourse.bacc as bacc
import concourse.tile as tile
import concourse.mybir as mybir
from concourse import bass_utils
from concourse.masks import make_identity

NCORES = 8
F = 64            # table row width (f32, 256B rows; cols 0..32 used)
FIN = 33          # x(32) + ones column
NW = 4            # table windows (int16 index limit)
CALL_MAX = 8192   # max indices per dma_gather call
TRACE = bool(os.environ.get("KERNEL_TRACE"))

LAST_EXEC_NS = []   # per-launch exec_time_ns when KERNEL_TRACE=1

if TRACE:
    # NTFF profiling shim: the image's antenv lacks axon_hooks, so register it
    # ourselves (dev-only; graders run with KERNEL_TRACE unset).
    import types

    if "antenv.axon_hooks" not in sys.modules:
        _m = types.ModuleType("antenv.axon_hooks")
        _m._hook = None
        _m.set_axon_ntff_profile_hook = lambda h: setattr(_m, "_hook", h)
        _m.get_axon_ntff_profile_hook = lambda: _m._hook
        sys.modules["antenv.axon_hooks"] = _m
        try:
            from trn_agent_boot.trn_boot import _ntff_profile_via_ctypes
            _m._hook = _ntff_profile_via_ctypes("/opt/axon/libaxon_pjrt.so")
        except Exception:
            _m._hook = None
    bass_utils.upload_artifacts = lambda tmpdir: f"local:{tmpdir}"

_prog_cache = {}


def _roundup(a, b):
    return (a + b - 1) // b * b


# ----------------------------------------------------------------- host plan
class Plan:
    pass


def build_plan(edge_index, batch, N, G):
    pl = Plan()
    src = np.asarray(edge_index[0], np.int64)
    dst = np.asarray(edge_index[1], np.int64)
    batch = np.asarray(batch, np.int64)
    gpc = G // NCORES  # graphs per core

    core_of_graph = np.arange(G) // gpc
    node_core = core_of_graph[batch]                      # [N]
    n0 = np.searchsorted(batch, np.arange(0, G, gpc))     # core node range start
    n1 = np.append(n0[1:], N)
    pl.n0, pl.n1, pl.gpc = n0, n1, gpc

    # graph-aligned local row layout: graph gi of a core starts at local row
    # (gi % gpc) * gmax128
    gstart = np.searchsorted(batch, np.arange(G))
    gsizes = np.bincount(batch, minlength=G)
    gmax128 = int(_roundup(max(int(gsizes.max()), 1), 128))
    NLpad = gpc * gmax128
    pl.gmax128, pl.NLpad = gmax128, NLpad

    lrow = (batch % gpc) * gmax128 + (np.arange(N) - gstart[batch])  # [N]
    pl.lrow = lrow
    pl.lrow2node = np.full((NCORES, NLpad), -1, np.int64)
    for c in range(NCORES):
        nn = np.arange(n0[c], n1[c])
        pl.lrow2node[c, lrow[nn]] = nn

    deg = np.bincount(dst, minlength=N)
    recip = np.where(deg > 0, 1.0 / np.maximum(deg, 1), 0.0).astype(np.float32)

    TL = NCORES * NLpad
    WROW = _roundup((TL + NW - 1) // NW, 128)
    assert WROW + 1 <= 32767, WROW
    pl.WROW = WROW

    # logical table row of node n
    tl = node_core[np.arange(N)] * NLpad + lrow
    pl.tl = tl

    src_w = tl[src] // WROW          # window of each edge's source
    src_li = tl[src] - src_w * WROW  # local row within window

    # per-core local edges
    dst_core = node_core[dst]
    ecore = [np.nonzero(dst_core == c)[0] for c in range(NCORES)]

    # per core, per window: CSR of edges grouped by local dst row, and the
    # node order sorted by in-window degree (desc). Groups of 128 nodes in
    # that order share one slot count K = max degree in the group (over cores).
    pl.win_nodes = []   # [c][w] -> local dst rows with deg_w>0, degree-sorted
    pl.win_adj = []     # [c][w] -> (uniq, starts, liw) CSR arrays
    profs = [[None] * NW for _ in range(NCORES)]
    for c in range(NCORES):
        e = ecore[c]
        ld = lrow[dst[e]]
        w = src_w[e]
        li = src_li[e]
        pl.win_nodes.append([])
        pl.win_adj.append([])
        for wi in range(NW):
            m = w == wi
            ldw, liw = ld[m], li[m]
            # sort by (dst row, src row): src-sorted slot runs improve DRAM
            # locality of the gather reads
            order = np.lexsort((liw, ldw))
            ldw, liw = ldw[order], liw[order]
            uniq, counts = np.unique(ldw, return_counts=True)
            # sort nodes by degree desc (stable in row id)
            no = np.argsort(-counts, kind='stable')
            pl.win_nodes[c].append(uniq[no])
            profs[c][wi] = counts[no]
            starts = np.concatenate([[0], np.cumsum(counts)])
            pl.win_adj[c].append((uniq, starts, liw))

    # per window: group count and per-group K (max over cores)
    pl.Ks = []          # [w] -> list of K per group
    for wi in range(NW):
        mx = max(len(profs[c][wi]) for c in range(NCORES))
        ng = (mx + 127) // 128
        P = np.zeros((NCORES, ng * 128), np.int64)
        for c in range(NCORES):
            P[c, :len(profs[c][wi])] = profs[c][wi]
        Ks = [int(P[:, g * 128:(g + 1) * 128].max()) for g in range(ng)]
        pl.Ks.append(Ks)

    # static call list: runs of equal-K groups, capped at CALL_MAX idxs
    calls = []
    sbase = 0
    icol = 0
    for wi in range(NW):
        Ks = pl.Ks[wi]
        g = 0
        while g < len(Ks):
            K = Ks[g]
            assert 1 <= K and 128 * K <= CALL_MAX, K
            cap = CALL_MAX // (128 * K)
            run = 1
            while run < cap and g + run < len(Ks) and Ks[g + run] == K:
                run += 1
            ni = run * K * 128
            calls.append(dict(w=wi, K=K, G=run, g0=g, sbase=sbase,
                              icol=icol, ni=ni))
            sbase += run * 128
            icol += ni // 16
            g += run
    pl.calls = calls
    pl.STOT = sbase
    pl.IDXC = icol

    # per-core idx buffer + recip buffer + S-row -> local-row map
    pl.idxbuf = np.full((NCORES, 128, pl.IDXC), WROW, np.int16)
    pl.recbuf = np.zeros((NCORES, 128, pl.STOT // 128), np.float32)
    pl.srow_node = np.full((NCORES, pl.STOT), -1, np.int64)  # local row or -1
    for c in range(NCORES):
        for call in calls:
            wi, K, Gc, g0 = call['w'], call['K'], call['G'], call['g0']
            nodes = pl.win_nodes[c][wi]
            uniq, starts, liw = pl.win_adj[c][wi]
            rank0 = g0 * 128
            r = max(0, min(len(nodes) - rank0, Gc * 128))
            L = np.full((Gc * K * 128,), pl.WROW, np.int16)  # default zero row
            if r > 0:
                nb = nodes[rank0:rank0 + r]
                t = np.arange(r)
                pl.srow_node[c, call['sbase'] + t] = nb
                ui = np.searchsorted(uniq, nb)
                d = (starts[ui + 1] - starts[ui]).astype(np.int64)
                assert d.max(initial=0) <= K
                tot = int(d.sum())
                tt = np.repeat(t, d)
                off = np.concatenate([[0], np.cumsum(d)[:-1]])
                jj = np.arange(tot) - np.repeat(off, d)
                L[((tt // 128) * K + jj) * 128 + (tt % 128)] = \
                    liw[np.repeat(starts[ui], d) + jj]
            ni = call['ni']
            Lw = L.reshape(ni // 16, 16).T  # [16, ni/16]
            pl.idxbuf[c, :, call['icol']:call['icol'] + ni // 16] = \
                np.tile(Lw, (8, 1))
    # real recip values
    for c in range(NCORES):
        rows = np.nonzero(pl.srow_node[c] >= 0)[0]
        gnodes = pl.lrow2node[c, pl.srow_node[c, rows]]
        assert (gnodes >= 0).all()
        pl.recbuf[c, rows % 128, rows // 128] = recip[gnodes]

    # per-core pooling mask: 0 on real rows, -1e30 on pad rows
    NB = NLpad // 128
    pl.poolmask = np.full((NCORES, 128, NB), -1e30, np.float32)
    for c in range(NCORES):
        rr = np.nonzero(pl.lrow2node[c] >= 0)[0]
        pl.poolmask[c, rr % 128, rr // 128] = 0.0
    return pl


def plan_from_inputs(edge_index, batch):
    return build_plan(edge_index, batch, batch.shape[0], int(batch.max()) + 1)


# ----------------------------------------------------------- device programs
def prog_agg(pl):
    key = ('agg', pl.STOT, pl.IDXC, len(pl.calls), pl.WROW)
    if key in _prog_cache:
        return _prog_cache[key]
    nc = bacc.Bacc("TRN2", target_bir_lowering=False, debug=False,
                   num_devices=NCORES, num_swdge_queues=4)
    tabs = [nc.dram_tensor(f"tab{w}", (pl.WROW + 1, F), mybir.dt.float32,
                           kind="ExternalInput").ap() for w in range(NW)]
    idx = nc.dram_tensor("idx", (128, pl.IDXC), mybir.dt.int16,
                         kind="ExternalInput").ap()
    rec = nc.dram_tensor("rec", (128, pl.STOT // 128), mybir.dt.float32,
                         kind="ExternalInput").ap()
    S = nc.dram_tensor("S", (pl.STOT, F), mybir.dt.float32,
                       kind="ExternalOutput").ap()

    with tile.TileContext(nc) as tc:
        with tc.tile_pool(name="io", bufs=1) as iop, \
             tc.tile_pool(name="g", bufs=6) as gp, \
             tc.tile_pool(name="st", bufs=4) as stp:
            idx_t = iop.tile([128, pl.IDXC], mybir.dt.int16)
            nc.sync.dma_start(out=idx_t[:], in_=idx[:, :])
            rec_t = iop.tile([128, pl.STOT // 128], mybir.dt.float32)
            nc.sync.dma_start(out=rec_t[:], in_=rec[:, :])
            for ci, call in enumerate(pl.calls):
                wi, K, Gc, ni = call['w'], call['K'], call['G'], call['ni']
                icol, sbase = call['icol'], call['sbase']
                t = gp.tile([128, Gc * K * F], mybir.dt.float32, tag="g")
                nc.gpsimd.dma_gather(
                    out_ap=t[:].rearrange("p (b f) -> p b f", f=F),
                    in_ap=tabs[wi][:],
                    idxs_ap=idx_t[:, icol:icol + ni // 16],
                    num_idxs=ni, num_idxs_reg=ni, elem_size=F,
                    single_packet=False, queue_num=ci % 4)
                tv = t[:].rearrange("p (g k f) -> p g k f", g=Gc, k=K)
                kk = K
                while kk > 1:
                    h = kk // 2
                    nc.vector.tensor_add(
                        out=tv[:, :, :h, :],
                        in0=tv[:, :, :h, :],
                        in1=tv[:, :, h:2 * h, :])
                    if kk % 2 == 1:
                        nc.vector.tensor_add(
                            out=tv[:, :, 0, :],
                            in0=tv[:, :, 0, :],
                            in1=tv[:, :, kk - 1, :])
                    kk = h
                stg = stp.tile([128, Gc * F], mybir.dt.float32, tag="st")
                rbc = rec_t[:, sbase // 128:sbase // 128 + Gc]
                nc.vector.tensor_mul(
                    out=stg[:].rearrange("p (g f) -> p g f", f=F),
                    in0=tv[:, :, 0, :],
                    in1=rbc.unsqueeze(2).broadcast_to([128, Gc, F]))
                nc.sync.dma_start(
                    out=S[sbase:sbase + Gc * 128, :].rearrange(
                        "(g p) f -> p g f", p=128),
                    in_=stg[:].rearrange("p (g f) -> p g f", f=F))
    nc.compile()
    _prog_cache[key] = nc
    return nc


def prog_comb(pl):
    key = ('comb', pl.NLpad)
    if key in _prog_cache:
        return _prog_cache[key]
    nc = bacc.Bacc("TRN2", target_bir_lowering=False, debug=False,
                   num_devices=NCORES)
    NB = pl.NLpad // 128
    Ss = [nc.dram_tensor(f"S{w}", (pl.NLpad, F), mybir.dt.float32,
                         kind="ExternalInput").ap() for w in range(NW)]
    y = nc.dram_tensor("y", (pl.NLpad, F), mybir.dt.float32,
                       kind="ExternalOutput").ap()
    with tile.TileContext(nc) as tc:
        with tc.tile_pool(name="p", bufs=1) as pp:
            ts = []
            for w in range(NW):
                t = pp.tile([128, NB * F], mybir.dt.float32, tag=f"s{w}")
                nc.sync.dma_start(
                    out=t[:].rearrange("p (b f) -> p b f", f=F),
                    in_=Ss[w][:, :].rearrange("(b p) f -> p b f", p=128))
                ts.append(t)
            nc.vector.tensor_add(out=ts[0][:], in0=ts[0][:], in1=ts[1][:])
            nc.vector.tensor_add(out=ts[2][:], in0=ts[2][:], in1=ts[3][:])
            nc.vector.tensor_add(out=ts[0][:], in0=ts[0][:], in1=ts[2][:])
            nc.sync.dma_start(
                out=y[:, :].rearrange("(b p) f -> p b f", p=128),
                in_=ts[0][:].rearrange("p (b f) -> p b f", f=F))
    nc.compile()
    _prog_cache[key] = nc
    return nc


def prog_tail(pl):
    key = ('tail', pl.NLpad, pl.gmax128, pl.gpc)
    if key in _prog_cache:
        return _prog_cache[key]
    nc = bacc.Bacc("TRN2", target_bir_lowering=False, debug=False,
                   num_devices=NCORES)
    NB = pl.NLpad // 128
    BPG = pl.gmax128 // 128  # blocks per graph
    GPC = pl.gpc
    Ss = [nc.dram_tensor(f"S{w}", (pl.NLpad, F), mybir.dt.float32,
                         kind="ExternalInput").ap() for w in range(NW)]
    ys = [nc.dram_tensor(f"y{k}", (pl.NLpad, F), mybir.dt.float32,
                         kind="ExternalInput").ap() for k in range(3)]
    C = nc.dram_tensor("C", (4 * F, F), mybir.dt.float32,
                       kind="ExternalInput").ap()
    Wo = nc.dram_tensor("Wo", (F, 8), mybir.dt.float32,
                        kind="ExternalInput").ap()
    bo = nc.dram_tensor("bo", (8, 8), mybir.dt.float32,
                        kind="ExternalInput").ap()
    msk = nc.dram_tensor("msk", (128, NB), mybir.dt.float32,
                         kind="ExternalInput").ap()
    out = nc.dram_tensor("out", (8, 8), mybir.dt.float32,
                         kind="ExternalOutput").ap()

    with tile.TileContext(nc) as tc:
        with tc.tile_pool(name="big", bufs=1) as bigp, \
             tc.tile_pool(name="wk", bufs=3) as wk, \
             tc.tile_pool(name="ps", bufs=2, space="PSUM") as ps:
            Ct = bigp.tile([128, 2 * F], mybir.dt.float32)  # [[C0;C1],[C2;C3]]
            nc.sync.dma_start(
                out=Ct[:].rearrange("p (k f) -> p k f", f=F),
                in_=C[:, :].rearrange("(k p) f -> p k f", p=128))
            ident = bigp.tile([128, 128], mybir.dt.float32)
            make_identity(nc, ident[:])
            Wot = bigp.tile([F, 8], mybir.dt.float32)
            nc.sync.dma_start(out=Wot[:], in_=Wo[:, :])
            bot = bigp.tile([8, 8], mybir.dt.float32)
            nc.sync.dma_start(out=bot[:], in_=bo[:, :])
            mskt = bigp.tile([128, NB], mybir.dt.float32)
            nc.sync.dma_start(out=mskt[:], in_=msk[:, :])
            acc = bigp.tile([128, GPC * F], mybir.dt.float32)
            nc.vector.memset(acc[:], -1e30)

            # h3 blocks: pack [y0|y1] and [y2|y3] pairs so one [128,128]
            # transpose + one matmul with stacked C rows handles two terms.
            CB = 28
            for c0 in range(0, NB, CB):
                cb = min(CB, NB - c0)
                rows = slice(c0 * 128, (c0 + cb) * 128)
                p01 = wk.tile([128, CB * 128], mybir.dt.float32, tag="p01")
                p23 = wk.tile([128, CB * 128], mybir.dt.float32, tag="p23")
                pv01 = p01[:].rearrange("p (b t) -> p b t", t=128)
                pv23 = p23[:].rearrange("p (b t) -> p b t", t=128)
                nc.sync.dma_start(
                    out=pv01[:, :cb, 0:F],
                    in_=ys[0][rows, :].rearrange("(b p) f -> p b f", p=128))
                nc.sync.dma_start(
                    out=pv01[:, :cb, F:128],
                    in_=ys[1][rows, :].rearrange("(b p) f -> p b f", p=128))
                nc.sync.dma_start(
                    out=pv23[:, :cb, 0:F],
                    in_=ys[2][rows, :].rearrange("(b p) f -> p b f", p=128))
                nc.sync.dma_start(
                    out=pv23[:, :cb, F:128],
                    in_=Ss[0][rows, :].rearrange("(b p) f -> p b f", p=128))
                for w in range(1, NW):
                    t = wk.tile([128, CB * F], mybir.dt.float32, tag="sw")
                    nc.sync.dma_start(
                        out=t[:, :cb * F].rearrange("p (b f) -> p b f", f=F),
                        in_=Ss[w][rows, :].rearrange("(b p) f -> p b f", p=128))
                    nc.vector.tensor_add(
                        out=pv23[:, :cb, F:128], in0=pv23[:, :cb, F:128],
                        in1=t[:, :cb * F].rearrange("p (b f) -> p b f", f=F))
                for blk in range(cb):
                    hp = ps.tile([128, F], mybir.dt.float32, space="PSUM",
                                 tag="hp")
                    for half, pv in ((0, pv01), (1, pv23)):
                        tp = ps.tile([128, 128], mybir.dt.float32,
                                     space="PSUM", tag="tp")
                        nc.tensor.transpose(out=tp[:], in_=pv[:, blk, :],
                                            identity=ident[:])
                        ykT = wk.tile([128, 128], mybir.dt.float32, tag="ykT")
                        nc.scalar.copy(out=ykT[:], in_=tp[:])
                        nc.tensor.matmul(
                            out=hp[:], lhsT=ykT[:],
                            rhs=Ct[:, half * F:(half + 1) * F],
                            start=(half == 0), stop=(half == 1))
                    b = c0 + blk
                    h3b = wk.tile([128, F], mybir.dt.float32, tag="h3b")
                    # mask pad rows to -1e30 while copying out of PSUM
                    nc.vector.tensor_scalar(
                        out=h3b[:], in0=hp[:], scalar1=mskt[:, b:b + 1],
                        scalar2=None, op0=mybir.AluOpType.add)
                    g = b // BPG
                    nc.vector.tensor_tensor(
                        out=acc[:, g * F:(g + 1) * F],
                        in0=acc[:, g * F:(g + 1) * F],
                        in1=h3b[:], op=mybir.AluOpType.max)

            # per graph: transpose [128,F] -> [F,128], tree-max over free dim
            pooledT = wk.tile([F, 8], mybir.dt.float32, tag="pt")
            for g in range(GPC):
                tp2 = ps.tile([F, 128], mybir.dt.float32, space="PSUM",
                              tag="tp")
                nc.tensor.transpose(out=tp2[:], in_=acc[:, g * F:(g + 1) * F],
                                    identity=ident[:])
                pc = wk.tile([F, 128], mybir.dt.float32, tag="pc")
                nc.scalar.copy(out=pc[:], in_=tp2[:])
                cc = 128
                while cc > 1:
                    h = cc // 2
                    nc.vector.tensor_tensor(
                        out=pc[:, :h], in0=pc[:, :h],
                        in1=pc[:, h:2 * h], op=mybir.AluOpType.max)
                    cc = h
                nc.vector.tensor_copy(out=pooledT[:, g:g + 1], in_=pc[:, :1])
            # logits = pooled @ Wo + bo
            lg = ps.tile([8, 8], mybir.dt.float32, space="PSUM", tag="lg")
            nc.tensor.matmul(out=lg[:], lhsT=pooledT[:], rhs=Wot[:],
                             start=True, stop=True)
            lgs = wk.tile([8, 8], mybir.dt.float32, tag="lgs")
            nc.vector.tensor_add(out=lgs[:], in0=lg[:], in1=bot[:])
            # log_softmax along free dim
            mx = wk.tile([8, 1], mybir.dt.float32, tag="mx")
            nc.vector.tensor_reduce(out=mx[:], in_=lgs[:],
                                    axis=mybir.AxisListType.X,
                                    op=mybir.AluOpType.max)
            nc.vector.tensor_scalar(out=lgs[:], in0=lgs[:], scalar1=mx[:, :1],
                                    scalar2=None,
                                    op0=mybir.AluOpType.subtract)
            ex = wk.tile([8, 8], mybir.dt.float32, tag="ex")
            nc.scalar.activation(out=ex[:], in_=lgs[:],
                                 func=mybir.ActivationFunctionType.Exp)
            sm = wk.tile([8, 1], mybir.dt.float32, tag="sm")
            nc.vector.tensor_reduce(out=sm[:], in_=ex[:],
                                    axis=mybir.AxisListType.X,
                                    op=mybir.AluOpType.add)
            lns = wk.tile([8, 1], mybir.dt.float32, tag="lns")
            nc.scalar.activation(out=lns[:], in_=sm[:],
                                 func=mybir.ActivationFunctionType.Ln)
            nc.vector.tensor_scalar(out=lgs[:], in0=lgs[:], scalar1=lns[:, :1],
                                    scalar2=None,
                                    op0=mybir.AluOpType.subtract)
            nc.sync.dma_start(out=out[:, :], in_=lgs[:])
    nc.compile()
    _prog_cache[key] = nc
    return nc


# ----------------------------------------------------------------- execution
def _run(nc, in_maps):
    res = bass_utils.run_bass_kernel_spmd(nc, in_maps,
                                          core_ids=list(range(NCORES)),
                                          trace=TRACE)
    if TRACE:
        LAST_EXEC_NS.append(res.exec_time_ns)
    return res.results


def _tables_from_y(pl, ylocal):
    """ylocal: [NCORES, NLpad, F] -> 4 window tables [WROW+1, F] (shared
    logical row space; per-core identical)."""
    TLrows = NCORES * pl.NLpad
    flat = np.zeros((NW * (pl.WROW + 1), F), np.float32)
    full = ylocal.reshape(TLrows, F)
    for w in range(NW):
        lo = w * pl.WROW
        hi = min(lo + pl.WROW, TLrows)
        flat[w * (pl.WROW + 1):w * (pl.WROW + 1) + (hi - lo)] = full[lo:hi]
    return [flat[w * (pl.WROW + 1):(w + 1) * (pl.WROW + 1)] for w in range(NW)]


def _align_partials(pl, Sout):
    """Sout: [NCORES, STOT, F] window-ordered partial sums -> aligned
    [NW, NCORES, NLpad, F] (host permutation only)."""
    out = np.zeros((NW, NCORES, pl.NLpad, F), np.float32)
    # S rows -> (window, local row) via call list
    for call in pl.calls:
        wi = call['w']
        rows = np.arange(call['sbase'], call['sbase'] + call['G'] * 128)
        for c in range(NCORES):
            nodes = pl.srow_node[c, rows]
            m = nodes >= 0
            out[wi, c, nodes[m]] = Sout[c, rows[m]]
    return out


def kernel(**inputs):
    x = np.asarray(inputs['x'], np.float32)
    edge_index = np.asarray(inputs['edge_index'])
    batch = np.asarray(inputs['batch'])
    N = x.shape[0]
    G = int(batch.max()) + 1
    pl = build_plan(edge_index, batch, N, G)

    # folded coefficient matrices (weights only)
    Wl = [np.asarray(inputs[f'Wl{i}'], np.float64) for i in range(3)]
    Wr = [np.asarray(inputs[f'Wr{i}'], np.float64) for i in range(3)]
    bl = [np.asarray(inputs[f'bl{i}'], np.float64) for i in range(3)]
    C0 = Wr[0] @ Wr[1] @ Wr[2]
    C1 = Wr[0] @ Wr[1] @ Wl[2] + Wr[0] @ Wl[1] @ Wr[2] + Wl[0] @ Wr[1] @ Wr[2]
    C2 = Wr[0] @ Wl[1] @ Wl[2] + Wl[0] @ Wr[1] @ Wl[2] + Wl[0] @ Wl[1] @ Wr[2]
    C3 = Wl[0] @ Wl[1] @ Wl[2]
    d0 = bl[0] @ Wr[1] @ Wr[2] + bl[1] @ Wr[2] + bl[2]
    d1 = bl[0] @ (Wr[1] @ Wl[2] + Wl[1] @ Wr[2]) + bl[1] @ Wl[2]
    d2 = bl[0] @ Wl[1] @ Wl[2]
    d3 = np.zeros(64)
    Cs = []
    for Cm, dv in [(C0, d0), (C1, d1), (C2, d2), (C3, d3)]:
        Cp = np.zeros((F, F), np.float32)
        Cp[:32] = Cm
        Cp[32] = dv
        Cs.append(Cp)
    Cstack = np.concatenate(Cs, axis=0)  # [4*64, 64]

    # y0 local: [NCORES, NLpad, 64], cols 0..31 = x, col 32 = 1 (real rows)
    y0 = np.zeros((NCORES, pl.NLpad, F), np.float32)
    for c in range(NCORES):
        rr = np.nonzero(pl.lrow2node[c] >= 0)[0]
        y0[c, rr, :32] = x[pl.lrow2node[c, rr]]
        y0[c, rr, 32] = 1.0

    nc_agg = prog_agg(pl)
    nc_comb = prog_comb(pl)

    ys = [y0]
    ycur = y0
    for _ in range(3):
        tabs = _tables_from_y(pl, ycur)
        in_maps = []
        for c in range(NCORES):
            m = {f"tab{w}": tabs[w] for w in range(NW)}
            m["idx"] = pl.idxbuf[c]
            m["rec"] = pl.recbuf[c]
            in_maps.append(m)
        res = _run(nc_agg, in_maps)
        Sout = np.stack([res[c]["S"] for c in range(NCORES)])
        parts = _align_partials(pl, Sout)
        if len(ys) < 3:
            in_maps = [{f"S{w}": parts[w, c] for w in range(NW)}
                       for c in range(NCORES)]
            res = _run(nc_comb, in_maps)
            ycur = np.stack([res[c]["y"] for c in range(NCORES)])
            ys.append(ycur)
        else:
            last_parts = parts
            break

    nc_tail = prog_tail(pl)
    bo = np.asarray(inputs['b_out'], np.float32)[None, :].repeat(8, axis=0)
    Wo = np.zeros((F, 8), np.float32)
    Wo[:] = np.asarray(inputs['W_out'], np.float32)
    in_maps = []
    for c in range(NCORES):
        m = {f"S{w}": last_parts[w, c] for w in range(NW)}
        for k in range(3):
            m[f"y{k}"] = ys[k][c]
        m["C"] = Cstack
        m["Wo"] = Wo
        m["bo"] = bo
        m["msk"] = pl.poolmask[c]
        in_maps.append(m)
    res = _run(nc_tail, in_maps)

    gpc = pl.gpc
    out = np.zeros((G, 8), np.float32)
    for c in range(NCORES):
        out[c * gpc:(c + 1) * gpc] = res[c]["out"]
    return out


# revision 10
# speedup vs baseline: 1.1566x; 1.1175x over previous
"""Trainium2 Bass kernel for nn_ClusterNet (3-layer linear GraphSAGE + max-pool + log_softmax).

Strategy
--------
The network is linear up to the final log_softmax:
    h3 = sum_{k=0..3} (M^k xt) Ct_k,   xt = [x | 1]  (bias column rides along),
where M = D^-1 A is the mean-aggregation operator and Ct_k are host-folded
products of the small weight matrices (weights are replicated; folding them is
constant preprocessing).

Device work (8 NeuronCores, graph-sharded by batch-id ranges so aggregation
output rows and pooling are core-local):
  - 3 rounds of "apply M": gather neighbor rows via nc.gpsimd.dma_gather from
    f32 tables (int16 indices -> 4 table windows). Nodes are sorted by
    in-degree (desc) and packed into 128-row groups; each group's slot count
    K is the max degree in the group, so padding is ~4% (vs 21% for
    power-of-two degree buckets). A strided tree-add reduces each node's K
    slots, then a recip scale. Gather calls rotate across all 4 SWDGE queues
    so Q7 descriptor generation and the SDMA transfers pipeline (~3.2 ns/idx
    vs ~9.3 single-queue).
  - window partials are recombined by a tiny add/scale launch; the host only
    PERMUTES rows between launches (no arithmetic on tensor data).
  - tail launch: h3 = sum_k y_k @ Ct_k via PE (transpose + accumulating
    matmuls). Local rows are laid out so each graph starts at a 128-aligned
    offset (gmax128 stride): the block->graph map is static and shared by all
    cores; a per-core additive mask (-1e30 on pad rows) makes the per-block
    partition-wise running max correct. One PE transpose + free-dim tree-max
    per graph, then logits and log_softmax on-device. Output [8 graphs, 8]
    per core, assembled on host.
"""
import os
import sys

sys.path.insert(0, '/opt/trn_rl_repo')

import numpy as np

import concourse.bass as bass
import concourse.bacc as bacc
import concourse.tile as tile
import concourse.mybir as mybir
from concourse import bass_utils
from concourse.masks import make_identity

NCORES = 8
F = 64            # table row width (f32, 256B rows; cols 0..32 used)
FIN = 33          # x(32) + ones column
NW = 4            # table windows (int16 index limit)
CALL_MAX = 4096   # max indices per dma_gather call
TRACE = bool(os.environ.get("KERNEL_TRACE"))

LAST_EXEC_NS = []   # per-launch exec_time_ns when KERNEL_TRACE=1

if TRACE:
    # NTFF profiling shim: the image's antenv lacks axon_hooks, so register it
    # ourselves (dev-only; graders run with KERNEL_TRACE unset).
    import types

    if "antenv.axon_hooks" not in sys.modules:
        _m = types.ModuleType("antenv.axon_hooks")
        _m._hook = None
        _m.set_axon_ntff_profile_hook = lambda h: setattr(_m, "_hook", h)
        _m.get_axon_ntff_profile_hook = lambda: _m._hook
        sys.modules["antenv.axon_hooks"] = _m
        try:
            from trn_agent_boot.trn_boot import _ntff_profile_via_ctypes
            _m._hook = _ntff_profile_via_ctypes("/opt/axon/libaxon_pjrt.so")
        except Exception:
            _m._hook = None
    bass_utils.upload_artifacts = lambda tmpdir: f"local:{tmpdir}"

_prog_cache = {}


def _roundup(a, b):
    return (a + b - 1) // b * b


# ----------------------------------------------------------------- host plan
class Plan:
    pass


def build_plan(edge_index, batch, N, G):
    pl = Plan()
    src = np.asarray(edge_index[0], np.int64)
    dst = np.asarray(edge_index[1], np.int64)
    batch = np.asarray(batch, np.int64)
    gpc = G // NCORES  # graphs per core

    core_of_graph = np.arange(G) // gpc
    node_core = core_of_graph[batch]                      # [N]
    n0 = np.searchsorted(batch, np.arange(0, G, gpc))     # core node range start
    n1 = np.append(n0[1:], N)
    pl.n0, pl.n1, pl.gpc = n0, n1, gpc

    # graph-aligned local row layout: graph gi of a core starts at local row
    # (gi % gpc) * gmax128
    gstart = np.searchsorted(batch, np.arange(G))
    gsizes = np.bincount(batch, minlength=G)
    gmax128 = int(_roundup(max(int(gsizes.max()), 1), 128))
    NLpad = gpc * gmax128
    pl.gmax128, pl.NLpad = gmax128, NLpad

    lrow = (batch % gpc) * gmax128 + (np.arange(N) - gstart[batch])  # [N]
    pl.lrow = lrow
    pl.lrow2node = np.full((NCORES, NLpad), -1, np.int64)
    for c in range(NCORES):
        nn = np.arange(n0[c], n1[c])
        pl.lrow2node[c, lrow[nn]] = nn

    deg = np.bincount(dst, minlength=N)
    recip = np.where(deg > 0, 1.0 / np.maximum(deg, 1), 0.0).astype(np.float32)

    TL = NCORES * NLpad
    WROW = _roundup((TL + NW - 1) // NW, 128)
    assert WROW + 1 <= 32767, WROW
    pl.WROW = WROW

    # logical table row of node n
    tl = node_core[np.arange(N)] * NLpad + lrow
    pl.tl = tl

    src_w = tl[src] // WROW          # window of each edge's source
    src_li = tl[src] - src_w * WROW  # local row within window

    # per-core local edges
    dst_core = node_core[dst]
    ecore = [np.nonzero(dst_core == c)[0] for c in range(NCORES)]

    # per core, per window: CSR of edges grouped by local dst row, and the
    # node order sorted by in-window degree (desc). Groups of 128 nodes in
    # that order share one slot count K = max degree in the group (over cores).
    pl.win_nodes = []   # [c][w] -> local dst rows with deg_w>0, degree-sorted
    pl.win_adj = []     # [c][w] -> (uniq, starts, liw) CSR arrays
    profs = [[None] * NW for _ in range(NCORES)]
    for c in range(NCORES):
        e = ecore[c]
        ld = lrow[dst[e]]
        w = src_w[e]
        li = src_li[e]
        pl.win_nodes.append([])
        pl.win_adj.append([])
        for wi in range(NW):
            m = w == wi
            ldw, liw = ld[m], li[m]
            # sort by (dst row, src row): src-sorted slot runs improve DRAM
            # locality of the gather reads
            order = np.lexsort((liw, ldw))
            ldw, liw = ldw[order], liw[order]
            uniq, counts = np.unique(ldw, return_counts=True)
            # sort nodes by degree desc (stable in row id)
            no = np.argsort(-counts, kind='stable')
            pl.win_nodes[c].append(uniq[no])
            profs[c][wi] = counts[no]
            starts = np.concatenate([[0], np.cumsum(counts)])
            pl.win_adj[c].append((uniq, starts, liw))

    # per window: group count and per-group K (max over cores)
    pl.Ks = []          # [w] -> list of K per group
    for wi in range(NW):
        mx = max(len(profs[c][wi]) for c in range(NCORES))
        ng = (mx + 127) // 128
        P = np.zeros((NCORES, ng * 128), np.int64)
        for c in range(NCORES):
            P[c, :len(profs[c][wi])] = profs[c][wi]
        Ks = [int(P[:, g * 128:(g + 1) * 128].max()) for g in range(ng)]
        pl.Ks.append(Ks)

    # static call list: runs of equal-K groups, capped at CALL_MAX idxs
    calls = []
    sbase = 0
    icol = 0
    for wi in range(NW):
        Ks = pl.Ks[wi]
        g = 0
        while g < len(Ks):
            K = Ks[g]
            assert 1 <= K and 128 * K <= CALL_MAX, K
            cap = CALL_MAX // (128 * K)
            run = 1
            while run < cap and g + run < len(Ks) and Ks[g + run] == K:
                run += 1
            ni = run * K * 128
            calls.append(dict(w=wi, K=K, G=run, g0=g, sbase=sbase,
                              icol=icol, ni=ni))
            sbase += run * 128
            icol += ni // 16
            g += run
    pl.calls = calls
    pl.STOT = sbase
    pl.IDXC = icol

    # per-core idx buffer + recip buffer + S-row -> local-row map
    pl.idxbuf = np.full((NCORES, 128, pl.IDXC), WROW, np.int16)
    pl.recbuf = np.zeros((NCORES, 128, pl.STOT // 128), np.float32)
    pl.srow_node = np.full((NCORES, pl.STOT), -1, np.int64)  # local row or -1
    for c in range(NCORES):
        for call in calls:
            wi, K, Gc, g0 = call['w'], call['K'], call['G'], call['g0']
            nodes = pl.win_nodes[c][wi]
            uniq, starts, liw = pl.win_adj[c][wi]
            rank0 = g0 * 128
            r = max(0, min(len(nodes) - rank0, Gc * 128))
            L = np.full((Gc * K * 128,), pl.WROW, np.int16)  # default zero row
            if r > 0:
                nb = nodes[rank0:rank0 + r]
                t = np.arange(r)
                pl.srow_node[c, call['sbase'] + t] = nb
                ui = np.searchsorted(uniq, nb)
                d = (starts[ui + 1] - starts[ui]).astype(np.int64)
                assert d.max(initial=0) <= K
                tot = int(d.sum())
                tt = np.repeat(t, d)
                off = np.concatenate([[0], np.cumsum(d)[:-1]])
                jj = np.arange(tot) - np.repeat(off, d)
                L[((tt // 128) * K + jj) * 128 + (tt % 128)] = \
                    liw[np.repeat(starts[ui], d) + jj]
            ni = call['ni']
            Lw = L.reshape(ni // 16, 16).T  # [16, ni/16]
            pl.idxbuf[c, :, call['icol']:call['icol'] + ni // 16] = \
                np.tile(Lw, (8, 1))
    # real recip values
    for c in range(NCORES):
        rows = np.nonzero(pl.srow_node[c] >= 0)[0]
        gnodes = pl.lrow2node[c, pl.srow_node[c, rows]]
        assert (gnodes >= 0).all()
        pl.recbuf[c, rows % 128, rows // 128] = recip[gnodes]

    # per-core pooling mask: 0 on real rows, -1e30 on pad rows
    NB = NLpad // 128
    pl.poolmask = np.full((NCORES, 128, NB), -1e30, np.float32)
    for c in range(NCORES):
        rr = np.nonzero(pl.lrow2node[c] >= 0)[0]
        pl.poolmask[c, rr % 128, rr // 128] = 0.0
    return pl


def plan_from_inputs(edge_index, batch):
    return build_plan(edge_index, batch, batch.shape[0], int(batch.max()) + 1)


# ----------------------------------------------------------- device programs
def prog_agg(pl):
    key = ('agg', pl.STOT, pl.IDXC, len(pl.calls), pl.WROW)
    if key in _prog_cache:
        return _prog_cache[key]
    nc = bacc.Bacc("TRN2", target_bir_lowering=False, debug=False,
                   num_devices=NCORES, num_swdge_queues=4)
    tabs = [nc.dram_tensor(f"tab{w}", (pl.WROW + 1, F), mybir.dt.float32,
                           kind="ExternalInput").ap() for w in range(NW)]
    idx = nc.dram_tensor("idx", (128, pl.IDXC), mybir.dt.int16,
                         kind="ExternalInput").ap()
    rec = nc.dram_tensor("rec", (128, pl.STOT // 128), mybir.dt.float32,
                         kind="ExternalInput").ap()
    S = nc.dram_tensor("S", (pl.STOT, F), mybir.dt.float32,
                       kind="ExternalOutput").ap()

    with tile.TileContext(nc) as tc:
        with tc.tile_pool(name="io", bufs=1) as iop, \
             tc.tile_pool(name="g", bufs=6) as gp, \
             tc.tile_pool(name="st", bufs=4) as stp:
            idx_t = iop.tile([128, pl.IDXC], mybir.dt.int16)
            nc.sync.dma_start(out=idx_t[:], in_=idx[:, :])
            rec_t = iop.tile([128, pl.STOT // 128], mybir.dt.float32)
            nc.sync.dma_start(out=rec_t[:], in_=rec[:, :])
            for ci, call in enumerate(pl.calls):
                wi, K, Gc, ni = call['w'], call['K'], call['G'], call['ni']
                icol, sbase = call['icol'], call['sbase']
                t = gp.tile([128, Gc * K * F], mybir.dt.float32, tag="g")
                nc.gpsimd.dma_gather(
                    out_ap=t[:].rearrange("p (b f) -> p b f", f=F),
                    in_ap=tabs[wi][:],
                    idxs_ap=idx_t[:, icol:icol + ni // 16],
                    num_idxs=ni, num_idxs_reg=ni, elem_size=F,
                    single_packet=False, queue_num=ci % 4)
                tv = t[:].rearrange("p (g k f) -> p g k f", g=Gc, k=K)
                kk = K
                while kk > 1:
                    h = kk // 2
                    nc.vector.tensor_add(
                        out=tv[:, :, :h, :],
                        in0=tv[:, :, :h, :],
                        in1=tv[:, :, h:2 * h, :])
                    if kk % 2 == 1:
                        nc.vector.tensor_add(
                            out=tv[:, :, 0, :],
                            in0=tv[:, :, 0, :],
                            in1=tv[:, :, kk - 1, :])
                    kk = h
                stg = stp.tile([128, Gc * F], mybir.dt.float32, tag="st")
                rbc = rec_t[:, sbase // 128:sbase // 128 + Gc]
                nc.vector.tensor_mul(
                    out=stg[:].rearrange("p (g f) -> p g f", f=F),
                    in0=tv[:, :, 0, :],
                    in1=rbc.unsqueeze(2).broadcast_to([128, Gc, F]))
                nc.sync.dma_start(
                    out=S[sbase:sbase + Gc * 128, :].rearrange(
                        "(g p) f -> p g f", p=128),
                    in_=stg[:].rearrange("p (g f) -> p g f", f=F))
    nc.compile()
    _prog_cache[key] = nc
    return nc


def prog_comb(pl):
    key = ('comb', pl.NLpad)
    if key in _prog_cache:
        return _prog_cache[key]
    nc = bacc.Bacc("TRN2", target_bir_lowering=False, debug=False,
                   num_devices=NCORES)
    NB = pl.NLpad // 128
    Ss = [nc.dram_tensor(f"S{w}", (pl.NLpad, F), mybir.dt.float32,
                         kind="ExternalInput").ap() for w in range(NW)]
    y = nc.dram_tensor("y", (pl.NLpad, F), mybir.dt.float32,
                       kind="ExternalOutput").ap()
    with tile.TileContext(nc) as tc:
        with tc.tile_pool(name="p", bufs=1) as pp:
            ts = []
            for w in range(NW):
                t = pp.tile([128, NB * F], mybir.dt.float32, tag=f"s{w}")
                nc.sync.dma_start(
                    out=t[:].rearrange("p (b f) -> p b f", f=F),
                    in_=Ss[w][:, :].rearrange("(b p) f -> p b f", p=128))
                ts.append(t)
            nc.vector.tensor_add(out=ts[0][:], in0=ts[0][:], in1=ts[1][:])
            nc.vector.tensor_add(out=ts[2][:], in0=ts[2][:], in1=ts[3][:])
            nc.vector.tensor_add(out=ts[0][:], in0=ts[0][:], in1=ts[2][:])
            nc.sync.dma_start(
                out=y[:, :].rearrange("(b p) f -> p b f", p=128),
                in_=ts[0][:].rearrange("p (b f) -> p b f", f=F))
    nc.compile()
    _prog_cache[key] = nc
    return nc


def prog_tail(pl):
    key = ('tail', pl.NLpad, pl.gmax128, pl.gpc)
    if key in _prog_cache:
        return _prog_cache[key]
    nc = bacc.Bacc("TRN2", target_bir_lowering=False, debug=False,
                   num_devices=NCORES)
    NB = pl.NLpad // 128
    BPG = pl.gmax128 // 128  # blocks per graph
    GPC = pl.gpc
    Ss = [nc.dram_tensor(f"S{w}", (pl.NLpad, F), mybir.dt.float32,
                         kind="ExternalInput").ap() for w in range(NW)]
    ys = [nc.dram_tensor(f"y{k}", (pl.NLpad, F), mybir.dt.float32,
                         kind="ExternalInput").ap() for k in range(3)]
    C = nc.dram_tensor("C", (4 * F, F), mybir.dt.float32,
                       kind="ExternalInput").ap()
    Wo = nc.dram_tensor("Wo", (F, 8), mybir.dt.float32,
                        kind="ExternalInput").ap()
    bo = nc.dram_tensor("bo", (8, 8), mybir.dt.float32,
                        kind="ExternalInput").ap()
    msk = nc.dram_tensor("msk", (128, NB), mybir.dt.float32,
                         kind="ExternalInput").ap()
    out = nc.dram_tensor("out", (8, 8), mybir.dt.float32,
                         kind="ExternalOutput").ap()

    with tile.TileContext(nc) as tc:
        with tc.tile_pool(name="big", bufs=1) as bigp, \
             tc.tile_pool(name="wk", bufs=3) as wk, \
             tc.tile_pool(name="ps", bufs=2, space="PSUM") as ps:
            Ct = bigp.tile([128, 2 * F], mybir.dt.float32)  # [[C0;C1],[C2;C3]]
            nc.sync.dma_start(
                out=Ct[:].rearrange("p (k f) -> p k f", f=F),
                in_=C[:, :].rearrange("(k p) f -> p k f", p=128))
            ident = bigp.tile([128, 128], mybir.dt.float32)
            make_identity(nc, ident[:])
            Wot = bigp.tile([F, 8], mybir.dt.float32)
            nc.sync.dma_start(out=Wot[:], in_=Wo[:, :])
            bot = bigp.tile([8, 8], mybir.dt.float32)
            nc.sync.dma_start(out=bot[:], in_=bo[:, :])
            mskt = bigp.tile([128, NB], mybir.dt.float32)
            nc.sync.dma_start(out=mskt[:], in_=msk[:, :])
            acc = bigp.tile([128, GPC * F], mybir.dt.float32)
            nc.vector.memset(acc[:], -1e30)

            # h3 blocks: pack [y0|y1] and [y2|y3] pairs so one [128,128]
            # transpose + one matmul with stacked C rows handles two terms.
            CB = 28
            for c0 in range(0, NB, CB):
                cb = min(CB, NB - c0)
                rows = slice(c0 * 128, (c0 + cb) * 128)
                p01 = wk.tile([128, CB * 128], mybir.dt.float32, tag="p01")
                p23 = wk.tile([128, CB * 128], mybir.dt.float32, tag="p23")
                pv01 = p01[:].rearrange("p (b t) -> p b t", t=128)
                pv23 = p23[:].rearrange("p (b t) -> p b t", t=128)
                nc.sync.dma_start(
                    out=pv01[:, :cb, 0:F],
                    in_=ys[0][rows, :].rearrange("(b p) f -> p b f", p=128))
                nc.sync.dma_start(
                    out=pv01[:, :cb, F:128],
                    in_=ys[1][rows, :].rearrange("(b p) f -> p b f", p=128))
                nc.sync.dma_start(
                    out=pv23[:, :cb, 0:F],
                    in_=ys[2][rows, :].rearrange("(b p) f -> p b f", p=128))
                nc.sync.dma_start(
                    out=pv23[:, :cb, F:128],
                    in_=Ss[0][rows, :].rearrange("(b p) f -> p b f", p=128))
                for w in range(1, NW):
                    t = wk.tile([128, CB * F], mybir.dt.float32, tag="sw")
                    nc.sync.dma_start(
                        out=t[:, :cb * F].rearrange("p (b f) -> p b f", f=F),
                        in_=Ss[w][rows, :].rearrange("(b p) f -> p b f", p=128))
                    nc.vector.tensor_add(
                        out=pv23[:, :cb, F:128], in0=pv23[:, :cb, F:128],
                        in1=t[:, :cb * F].rearrange("p (b f) -> p b f", f=F))
                for blk in range(cb):
                    hp = ps.tile([128, F], mybir.dt.float32, space="PSUM",
                                 tag="hp")
                    for half, pv in ((0, pv01), (1, pv23)):
                        tp = ps.tile([128, 128], mybir.dt.float32,
                                     space="PSUM", tag="tp")
                        nc.tensor.transpose(out=tp[:], in_=pv[:, blk, :],
                                            identity=ident[:])
                        ykT = wk.tile([128, 128], mybir.dt.float32, tag="ykT")
                        nc.scalar.copy(out=ykT[:], in_=tp[:])
                        nc.tensor.matmul(
                            out=hp[:], lhsT=ykT[:],
                            rhs=Ct[:, half * F:(half + 1) * F],
                            start=(half == 0), stop=(half == 1))
                    b = c0 + blk
                    h3b = wk.tile([128, F], mybir.dt.float32, tag="h3b")
                    # mask pad rows to -1e30 while copying out of PSUM
                    nc.vector.tensor_scalar(
                        out=h3b[:], in0=hp[:], scalar1=mskt[:, b:b + 1],
                        scalar2=None, op0=mybir.AluOpType.add)
                    g = b // BPG
                    nc.vector.tensor_tensor(
                        out=acc[:, g * F:(g + 1) * F],
                        in0=acc[:, g * F:(g + 1) * F],
                        in1=h3b[:], op=mybir.AluOpType.max)

            # per graph: transpose [128,F] -> [F,128], tree-max over free dim
            pooledT = wk.tile([F, 8], mybir.dt.float32, tag="pt")
            for g in range(GPC):
                tp2 = ps.tile([F, 128], mybir.dt.float32, space="PSUM",
                              tag="tp")
                nc.tensor.transpose(out=tp2[:], in_=acc[:, g * F:(g + 1) * F],
                                    identity=ident[:])
                pc = wk.tile([F, 128], mybir.dt.float32, tag="pc")
                nc.scalar.copy(out=pc[:], in_=tp2[:])
                cc = 128
                while cc > 1:
                    h = cc // 2
                    nc.vector.tensor_tensor(
                        out=pc[:, :h], in0=pc[:, :h],
                        in1=pc[:, h:2 * h], op=mybir.AluOpType.max)
                    cc = h
                nc.vector.tensor_copy(out=pooledT[:, g:g + 1], in_=pc[:, :1])
            # logits = pooled @ Wo + bo
            lg = ps.tile([8, 8], mybir.dt.float32, space="PSUM", tag="lg")
            nc.tensor.matmul(out=lg[:], lhsT=pooledT[:], rhs=Wot[:],
                             start=True, stop=True)
            lgs = wk.tile([8, 8], mybir.dt.float32, tag="lgs")
            nc.vector.tensor_add(out=lgs[:], in0=lg[:], in1=bot[:])
            # log_softmax along free dim
            mx = wk.tile([8, 1], mybir.dt.float32, tag="mx")
            nc.vector.tensor_reduce(out=mx[:], in_=lgs[:],
                                    axis=mybir.AxisListType.X,
                                    op=mybir.AluOpType.max)
            nc.vector.tensor_scalar(out=lgs[:], in0=lgs[:], scalar1=mx[:, :1],
                                    scalar2=None,
                                    op0=mybir.AluOpType.subtract)
            ex = wk.tile([8, 8], mybir.dt.float32, tag="ex")
            nc.scalar.activation(out=ex[:], in_=lgs[:],
                                 func=mybir.ActivationFunctionType.Exp)
            sm = wk.tile([8, 1], mybir.dt.float32, tag="sm")
            nc.vector.tensor_reduce(out=sm[:], in_=ex[:],
                                    axis=mybir.AxisListType.X,
                                    op=mybir.AluOpType.add)
            lns = wk.tile([8, 1], mybir.dt.float32, tag="lns")
            nc.scalar.activation(out=lns[:], in_=sm[:],
                                 func=mybir.ActivationFunctionType.Ln)
            nc.vector.tensor_scalar(out=lgs[:], in0=lgs[:], scalar1=lns[:, :1],
                                    scalar2=None,
                                    op0=mybir.AluOpType.subtract)
            nc.sync.dma_start(out=out[:, :], in_=lgs[:])
    nc.compile()
    _prog_cache[key] = nc
    return nc


# ----------------------------------------------------------------- execution
def _run(nc, in_maps):
    res = bass_utils.run_bass_kernel_spmd(nc, in_maps,
                                          core_ids=list(range(NCORES)),
                                          trace=TRACE)
    if TRACE:
        LAST_EXEC_NS.append(res.exec_time_ns)
    return res.results


def _tables_from_y(pl, ylocal):
    """ylocal: [NCORES, NLpad, F] -> 4 window tables [WROW+1, F] (shared
    logical row space; per-core identical)."""
    TLrows = NCORES * pl.NLpad
    flat = np.zeros((NW * (pl.WROW + 1), F), np.float32)
    full = ylocal.reshape(TLrows, F)
    for w in range(NW):
        lo = w * pl.WROW
        hi = min(lo + pl.WROW, TLrows)
        flat[w * (pl.WROW + 1):w * (pl.WROW + 1) + (hi - lo)] = full[lo:hi]
    return [flat[w * (pl.WROW + 1):(w + 1) * (pl.WROW + 1)] for w in range(NW)]


def _align_partials(pl, Sout):
    """Sout: [NCORES, STOT, F] window-ordered partial sums -> aligned
    [NW, NCORES, NLpad, F] (host permutation only)."""
    out = np.zeros((NW, NCORES, pl.NLpad, F), np.float32)
    # S rows -> (window, local row) via call list
    for call in pl.calls:
        wi = call['w']
        rows = np.arange(call['sbase'], call['sbase'] + call['G'] * 128)
        for c in range(NCORES):
            nodes = pl.srow_node[c, rows]
            m = nodes >= 0
            out[wi, c, nodes[m]] = Sout[c, rows[m]]
    return out


def kernel(**inputs):
    x = np.asarray(inputs['x'], np.float32)
    edge_index = np.asarray(inputs['edge_index'])
    batch = np.asarray(inputs['batch'])
    N = x.shape[0]
    G = int(batch.max()) + 1
    pl = build_plan(edge_index, batch, N, G)

    # folded coefficient matrices (weights only)
    Wl = [np.asarray(inputs[f'Wl{i}'], np.float64) for i in range(3)]
    Wr = [np.asarray(inputs[f'Wr{i}'], np.float64) for i in range(3)]
    bl = [np.asarray(inputs[f'bl{i}'], np.float64) for i in range(3)]
    C0 = Wr[0] @ Wr[1] @ Wr[2]
    C1 = Wr[0] @ Wr[1] @ Wl[2] + Wr[0] @ Wl[1] @ Wr[2] + Wl[0] @ Wr[1] @ Wr[2]
    C2 = Wr[0] @ Wl[1] @ Wl[2] + Wl[0] @ Wr[1] @ Wl[2] + Wl[0] @ Wl[1] @ Wr[2]
    C3 = Wl[0] @ Wl[1] @ Wl[2]
    d0 = bl[0] @ Wr[1] @ Wr[2] + bl[1] @ Wr[2] + bl[2]
    d1 = bl[0] @ (Wr[1] @ Wl[2] + Wl[1] @ Wr[2]) + bl[1] @ Wl[2]
    d2 = bl[0] @ Wl[1] @ Wl[2]
    d3 = np.zeros(64)
    Cs = []
    for Cm, dv in [(C0, d0), (C1, d1), (C2, d2), (C3, d3)]:
        Cp = np.zeros((F, F), np.float32)
        Cp[:32] = Cm
        Cp[32] = dv
        Cs.append(Cp)
    Cstack = np.concatenate(Cs, axis=0)  # [4*64, 64]

    # y0 local: [NCORES, NLpad, 64], cols 0..31 = x, col 32 = 1 (real rows)
    y0 = np.zeros((NCORES, pl.NLpad, F), np.float32)
    for c in range(NCORES):
        rr = np.nonzero(pl.lrow2node[c] >= 0)[0]
        y0[c, rr, :32] = x[pl.lrow2node[c, rr]]
        y0[c, rr, 32] = 1.0

    nc_agg = prog_agg(pl)
    nc_comb = prog_comb(pl)

    ys = [y0]
    ycur = y0
    for _ in range(3):
        tabs = _tables_from_y(pl, ycur)
        in_maps = []
        for c in range(NCORES):
            m = {f"tab{w}": tabs[w] for w in range(NW)}
            m["idx"] = pl.idxbuf[c]
            m["rec"] = pl.recbuf[c]
            in_maps.append(m)
        res = _run(nc_agg, in_maps)
        Sout = np.stack([res[c]["S"] for c in range(NCORES)])
        parts = _align_partials(pl, Sout)
        if len(ys) < 3:
            in_maps = [{f"S{w}": parts[w, c] for w in range(NW)}
                       for c in range(NCORES)]
            res = _run(nc_comb, in_maps)
            ycur = np.stack([res[c]["y"] for c in range(NCORES)])
            ys.append(ycur)
        else:
            last_parts = parts
            break

    nc_tail = prog_tail(pl)
    bo = np.asarray(inputs['b_out'], np.float32)[None, :].repeat(8, axis=0)
    Wo = np.zeros((F, 8), np.float32)
    Wo[:] = np.asarray(inputs['W_out'], np.float32)
    in_maps = []
    for c in range(NCORES):
        m = {f"S{w}": last_parts[w, c] for w in range(NW)}
        for k in range(3):
            m[f"y{k}"] = ys[k][c]
        m["C"] = Cstack
        m["Wo"] = Wo
        m["bo"] = bo
        m["msk"] = pl.poolmask[c]
        in_maps.append(m)
    res = _run(nc_tail, in_maps)

    gpc = pl.gpc
    out = np.zeros((G, 8), np.float32)
    for c in range(NCORES):
        out[c * gpc:(c + 1) * gpc] = res[c]["out"]
    return out
